# revision 18
# baseline (speedup 1.0000x reference)
# Trainium2 Bass kernel for nn_AttentionalPropagation (B=2, D=256, N=M=4096, H=4).
#
# Sharding: 8 cores; each batch (B=2) owns 4 cores; each core computes a
# 1024-column sequence shard of the output end-to-end. k,v are computed
# redundantly per core from the full `source` of its batch. The only
# cross-core communication is an AllGather of the InstanceNorm partial
# (sum, sumsq) statistics within each 4-core batch group.
#
# Bias algebra (host-side folding):
#   bk drops: adds a per-n constant to scores -> softmax-invariant over m.
#   bv drops: adds bv to attn (sum prob = 1) -> folded via W1m@Wm@bv ... but
#             b1 itself cancels in InstanceNorm (mean-subtracted), so the
#             whole msg/cat bias chain (bm, bv, b1) vanishes.
#   Wm folds into W1: h1 = W1x@x + (W1m@Wm)@attn.
#   Only bq and b2 survive.
#
# Precision plan:
#   q,k fp8e4 + DoubleRow scores matmul ([32,2,*] layout, contraction 64).
#   exp: split ACT (native Exp -> fp8) / DVE (Schraudolph bit-trick:
#        uint8 = RNE(score*log2e + 56) == fp8e4 bits of exp(score/8)).
#   v fp8; attn matmul fp8 DoubleRow with 64 ones-columns appended so rows
#   64..127 of the accumulator hold Z (softmax denom) replicated -> parallel
#   reciprocal.
#   Everything else bf16 (h1, W2), fp32 accumulation.

import os
import numpy as np

import concourse.bass as bass  # noqa: F401
import concourse.tile as tile
import concourse.mybir as mybir
from concourse import bacc
from concourse import bass_utils

B, D, N = 2, 256, 4096
H, DH = 4, 64
NS = N // 4           # sequence shard per core
NCORES = 8
EPS = 1e-5
LOG2E = 1.4426950408889634
EXPC = 56.0           # fp8e4 exponent-bias shift: bits = RNE(s*log2e + 56)

FP = mybir.dt.float32
BF = mybir.dt.bfloat16
F8 = mybir.dt.float8e4
U8 = mybir.dt.uint8
I32 = mybir.dt.int32
U32 = mybir.dt.uint32
AX = mybir.AxisListType
OP = mybir.AluOpType
AF = mybir.ActivationFunctionType
DR = mybir.MatmulPerfMode.DoubleRow

_STAGE = os.environ.get("KSTAGE", "full")  # proj|attn|h1|nocc|full

# Per-(h,nc) exp group -> engine assignment: 16 groups of 2 m-chunks.
# ~62% ACT / 38% DVE interleaved (DVE also carries the evac copies).
_EXP_ON_ACT = [True, True, False, True, True, False, True, True,
               False, True, True, False, True, True, False, True]


def _emit(nc, tc, io, es):
    wpool = es.enter_context(tc.tile_pool(name="weights", bufs=1))
    apool = es.enter_context(tc.tile_pool(name="acts", bufs=1))

    # ---------- warm-up: ACT exp table load + PE warm stream ----------
    warm = apool.tile([128, 8], FP)
    nc.vector.memset(warm[:], 0.0)
    nc.scalar.activation(out=warm[:, 0:4], in_=warm[:, 4:8], func=AF.Exp, scale=1.0)

    # ---------- weight / bias / input loads ----------
    wq_sb = wpool.tile([128, 2, D], BF)
    nc.sync.dma_start(out=wq_sb[:], in_=io["wqT"].rearrange("(c p) o -> p c o", p=128))
    wk_sb = wpool.tile([128, 2, D], BF)
    nc.sync.dma_start(out=wk_sb[:], in_=io["wkT"].rearrange("(c p) o -> p c o", p=128))
    wv_sb = wpool.tile([128, 2, D], BF)
    nc.sync.dma_start(out=wv_sb[:], in_=io["wvT"].rearrange("(c p) o -> p c o", p=128))
    w1x_sb = wpool.tile([128, 2, 2 * D], BF)
    nc.gpsimd.dma_start(out=w1x_sb[:], in_=io["w1xT"].rearrange("(c p) o -> p c o", p=128))
    w1m_sb = wpool.tile([128, 2, 2 * D], BF)
    nc.gpsimd.dma_start(out=w1m_sb[:], in_=io["w1mT"].rearrange("(c p) o -> p c o", p=128))
    w2_sb = wpool.tile([128, 4, D], BF)
    nc.gpsimd.dma_start(out=w2_sb[:], in_=io["w2T"].rearrange("(c p) o -> p c o", p=128))

    bq_sb = wpool.tile([128, 2], FP)
    nc.sync.dma_start(out=bq_sb[:], in_=io["bq"][:])
    b2_sb = wpool.tile([128, 2], FP)
    nc.sync.dma_start(out=b2_sb[:], in_=io["b2"][:])

    xs_sb = apool.tile([128, 2, NS], BF)
    nc.sync.dma_start(out=xs_sb[:], in_=io["xs"].rearrange("(c p) n -> p c n", p=128))

    # ---------- persistent activation tiles ----------
    qf8 = apool.tile([128, 2, NS], U8)
    kf8 = apool.tile([128, 2, N], U8)
    q8 = apool.tile([32, H, 2, NS], U8)
    k8 = apool.tile([32, H, 2, N], U8)
    # v^T per head + 64 ones cols (Z replication), fp8: [m-part, h, p16, j, 128]
    vaT = apool.tile([128, H, 16, 2, 128], F8)
    exp8 = apool.tile([128, 32, 512], U8)
    attn_sb = apool.tile([128, 2, NS], BF)
    h1_sb = apool.tile([128, 4, NS], BF)
    z_sb = apool.tile([128, 4, NS], BF)
    out_sb = apool.tile([128, 2, NS], FP)
    stats_sb = apool.tile([128, 8], FP)
    stats8 = apool.tile([128, 8, 8], FP)
    sq_sb = apool.tile([128, NS], FP)
    scrA = apool.tile([128, 512], FP)
    scrB = apool.tile([128, 256], FP)
    w2s_sb = apool.tile([128, 4, D], BF)

    # ones in cols 0..63 so Z lands at base partition 0 (custom-DVE recip
    # ignores input base_partition); v data in cols 64..127.
    for _h in range(H):
        for _j in range(2):
            nc.gpsimd.memset(vaT[:, _h, :, _j, 0:DH], 1.0)

    # ---------- phase 1: projections ----------
    with tc.tile_pool(name="srcp", bufs=1) as srcpool, \
         tc.tile_pool(name="pj", bufs=2, space="PSUM") as pj, \
         tc.tile_pool(name="vt", bufs=3, space="PSUM") as vtp:
        src_sb = srcpool.tile([128, 2, N], BF)
        nc.sync.dma_start(out=src_sb[:], in_=io["src"].rearrange("(c p) m -> p c m", p=128))

        # k = WkT.T @ src (no bias), per oc chunk; evac fp8, then per-head DMA
        # shuffle into the DoubleRow [32, 2, m] layout.
        for oc in range(2):
            for ns in range(N // 512 // 2):
                k_ps = pj.tile([128, 2, 512], FP, tag="pj")
                for half in range(2):
                    for ic in range(2):
                        nc.tensor.matmul(
                            k_ps[:, half, :],
                            wk_sb[:, ic, oc * 128:(oc + 1) * 128],
                            src_sb[:, ic, (ns * 2 + half) * 512:(ns * 2 + half + 1) * 512],
                            start=(ic == 0), stop=(ic == 1),
                        )
                nc.vector.tensor_copy(
                    kf8[:, oc, ns * 1024:(ns + 1) * 1024].bitcast(F8)
                    .rearrange("p (a n) -> p a n", a=2), k_ps[:])
            # shuffle: k8[p, h, j, m] = kf8[64*(h%2)+32*j+p, h//2, m]
            for m in range(2):
                h = 2 * oc + m
                for j in range(2):
                    nc.sync.dma_start(
                        out=k8[:, h, j, :],
                        in_=kf8[64 * m + 32 * j:64 * m + 32 * j + 32, oc, :])

        # q = WqT.T @ xs + bq; evac fp8 + shuffle
        for oc in range(2):
            q_ps = pj.tile([128, 2, 512], FP, tag="pj")
            for half in range(2):
                for ic in range(2):
                    nc.tensor.matmul(
                        q_ps[:, half, :],
                        wq_sb[:, ic, oc * 128:(oc + 1) * 128],
                        xs_sb[:, ic, half * 512:(half + 1) * 512],
                        start=(ic == 0), stop=(ic == 1),
                    )
            nc.vector.tensor_scalar_add(
                qf8[:, oc, :].bitcast(F8).rearrange("p (a n) -> p a n", a=2),
                q_ps[:], bq_sb[:, oc:oc + 1])
            for m in range(2):
                h = 2 * oc + m
                for j in range(2):
                    nc.sync.dma_start(
                        out=q8[:, h, j, :],
                        in_=qf8[64 * m + 32 * j:64 * m + 32 * j + 32, oc, :])

        # v^T directly transposed: out[m, c] = sum_i src[i, m] WvT[i, c]
        for mp in range(16):
            vt_ps = vtp.tile([128, 2, D], FP, tag="vt")
            for j in range(2):
                for ic in range(2):
                    nc.tensor.matmul(
                        vt_ps[:, j, :],
                        src_sb[:, ic, (2 * mp + j) * 128:(2 * mp + j + 1) * 128],
                        wv_sb[:, ic, :],
                        start=(ic == 0), stop=(ic == 1),
                    )
            # in order (j, h, d) -> out vaT[:, h, mp, j, 64:128] reordered (j, h, d)
            nc.vector.tensor_copy(
                vaT[:, :, mp, :, DH:128].rearrange("p h j d -> p j h d"),
                vt_ps[:].rearrange("p j (h d) -> p j h d", h=H))

    if _STAGE == "proj":
        nc.vector.tensor_copy(out_sb[:, 0, :], xs_sb[:, 0, :])
        nc.vector.tensor_copy(out_sb[:, 1, :], xs_sb[:, 1, :])
        nc.sync.dma_start(out=io["out"].rearrange("(c p) n -> p c n", p=128), in_=out_sb[:])
        return

    # ---------- phase 2+3 interleaved: attention then h1 per nc chunk ----------
    with tc.tile_pool(name="sc", bufs=1, space="PSUM") as scp, \
         tc.tile_pool(name="at", bufs=2, space="PSUM") as atp, \
         tc.tile_pool(name="mm", bufs=1, space="PSUM") as mmp, \
         tc.tile_pool(name="nrm", bufs=2) as nrm:
        sc = scp.tile([128, 4, 512], FP)  # quartered: 2-mc groups double-buffered
        for nch in range(2):
            n0 = nch * 512
            for h in range(4):
                at_ps = atp.tile([128, 512], FP, tag="at")
                for g in range(16):      # groups of 2 m-chunks
                    q2 = 2 * (g % 2)
                    for j2 in range(2):  # m-chunk within group
                        mc = 2 * g + j2
                        nc.tensor.matmul(
                            sc[:, q2 + j2, :],
                            k8[:, h, :, mc * 128:(mc + 1) * 128].bitcast(F8),
                            q8[:, h, :, n0:n0 + 512].bitcast(F8),
                            start=True, stop=True,
                            perf_mode=DR,
                        )
                    if _EXP_ON_ACT[g]:
                        nc.scalar.activation(
                            out=exp8[:, 2 * g:2 * g + 2, :].bitcast(F8),
                            in_=sc[:, q2:q2 + 2, :], func=AF.Exp, scale=0.125)
                    else:
                        nc.vector.tensor_scalar(
                            exp8[:, 2 * g:2 * g + 2, :],
                            sc[:, q2:q2 + 2, :], LOG2E, EXPC,
                            op0=OP.mult, op1=OP.add)
                    nc.tensor.matmul(
                        at_ps[:],
                        vaT[:, h, g, :, :],
                        exp8[:, 2 * g:2 * g + 2, :].bitcast(F8),
                        start=(g == 0), stop=(g == 15),
                        perf_mode=DR,
                    )
                rzb = nrm.tile([DH, 512], FP, tag="rzb")
                nc.vector.reciprocal_approx_fast(rzb[:], at_ps[0:DH, :])
                bp = 64 * (h % 2)
                nc.vector.tensor_tensor(
                    out=attn_sb[bp:bp + DH, h // 2, n0:n0 + 512],
                    in0=at_ps[DH:128, :], in1=rzb[:], op=OP.mult)

            if _STAGE == "attn" and nch == 1:
                nc.vector.tensor_copy(out_sb[:, 0, :], attn_sb[:, 0, :])
                nc.vector.tensor_copy(out_sb[:, 1, :], attn_sb[:, 1, :])
                nc.sync.dma_start(out=io["out"].rearrange("(c p) n -> p c n", p=128), in_=out_sb[:])
                return

            # h1 (raw, biasless) = W1x@xs + (W1m@Wm)@attn for this nc chunk
            for op2 in range(2):  # oc pairs
                h_ps = mmp.tile([128, 2, 512], FP, tag="mm")
                for oc2 in range(2):
                    oc = 2 * op2 + oc2
                    for ic in range(2):
                        nc.tensor.matmul(
                            h_ps[:, oc2, :],
                            w1x_sb[:, ic, oc * 128:(oc + 1) * 128],
                            xs_sb[:, ic, n0:n0 + 512],
                            start=(ic == 0), stop=False,
                        )
                    for ic in range(2):
                        nc.tensor.matmul(
                            h_ps[:, oc2, :],
                            w1m_sb[:, ic, oc * 128:(oc + 1) * 128],
                            attn_sb[:, ic, n0:n0 + 512],
                            start=False, stop=(ic == 1),
                        )
                nc.vector.tensor_copy(
                    h1_sb[:, 2 * op2:2 * op2 + 2, n0:n0 + 512], h_ps[:])

    if _STAGE == "h1":
        nc.vector.tensor_copy(out_sb[:, 0, :], h1_sb[:, 0, :])
        nc.vector.tensor_copy(out_sb[:, 1, :], h1_sb[:, 1, :])
        nc.sync.dma_start(out=io["out"].rearrange("(c p) n -> p c n", p=128), in_=out_sb[:])
        return

    # ---------- phase 4: stats, gather, norm-fold, W2 ----------
    with tc.tile_pool(name="dram", bufs=1, space="DRAM") as dram, \
         tc.tile_pool(name="nstat", bufs=1) as nstat, \
         tc.tile_pool(name="w2p", bufs=2, space="PSUM") as w2p:
        # per-core partial sums over local NS columns: Pool pairwise tree down
        # to width 8 (Pool has no free-dim reduce), then one DVE reduce.
        def colsum8(src, slot):
            add = lambda o, a, b: nc.gpsimd.tensor_tensor(out=o, in0=a, in1=b, op=OP.add)
            add(scrA[:, 0:256], src[:, 0:256], src[:, 256:512])
            add(scrA[:, 256:512], src[:, 512:768], src[:, 768:1024])
            add(scrB[:, 0:256], scrA[:, 0:256], scrA[:, 256:512])
            add(scrA[:, 0:64], scrB[:, 0:64], scrB[:, 64:128])
            add(scrA[:, 64:128], scrB[:, 128:192], scrB[:, 192:256])
            add(scrA[:, 128:192], scrA[:, 0:64], scrA[:, 64:128])
            add(scrB[:, 0:16], scrA[:, 128:144], scrA[:, 144:160])
            add(scrB[:, 16:32], scrA[:, 160:176], scrA[:, 176:192])
            add(scrB[:, 32:48], scrB[:, 0:16], scrB[:, 16:32])
            add(stats8[:, slot, :], scrB[:, 32:40], scrB[:, 40:48])

        for t in range(4):
            colsum8(h1_sb[:, t, :], t)
            nc.gpsimd.tensor_tensor(
                out=sq_sb[:], in0=h1_sb[:, t, :], in1=h1_sb[:, t, :], op=OP.mult)
            colsum8(sq_sb[:], 4 + t)
        nc.vector.tensor_reduce(out=stats_sb[:], in_=stats8[:], axis=AX.X, op=OP.add)

        sred = nstat.tile([128, 8], FP)
        if _STAGE == "nocc":
            nc.vector.tensor_scalar_mul(sred[:], stats_sb[:], 4.0)
        else:
            cc_in = dram.tile([128, 8], FP)
            cc_out = dram.tile([4, 128, 8], FP)
            nc.sync.dma_start(out=cc_in[:], in_=stats_sb[:])
            nc.gpsimd.collective_compute(
                "AllGather", OP.bypass,
                replica_groups=[[0, 1, 2, 3], [4, 5, 6, 7]],
                ins=[cc_in[:].opt()], outs=[cc_out[:].opt()],
            )
            gat = nstat.tile([128, 4, 8], FP)
            nc.sync.dma_start(out=gat[:], in_=cc_out[:].rearrange("s p f -> p s f"))
            s01 = nstat.tile([128, 2, 8], FP)
            nc.vector.tensor_add(s01[:], gat[:, 0:2, :], gat[:, 2:4, :])
            nc.vector.tensor_add(sred[:], s01[:, 0, :], s01[:, 1, :])

        # mu = sum/N ; var = sumsq/N - mu^2 ; rstd = rsqrt(var+eps) (Newton)
        mu4 = nstat.tile([128, 4], FP)
        nc.vector.tensor_scalar_mul(mu4[:], sred[:, 0:4], 1.0 / N)
        var4 = nstat.tile([128, 4], FP)
        nc.vector.tensor_scalar_mul(var4[:], sred[:, 4:8], 1.0 / N)
        musq = nstat.tile([128, 4], FP)
        nc.vector.tensor_mul(musq[:], mu4[:], mu4[:])
        nc.vector.tensor_tensor(out=var4[:], in0=var4[:], in1=musq[:], op=OP.subtract)
        nc.vector.tensor_scalar_add(var4[:], var4[:], EPS)
        # Newton rsqrt: y0 = bits(0x5F3759DF - (u>>1)); y_{k+1} = y(1.5 - 0.5 x y^2)
        sh = nstat.tile([128, 4], I32)
        nc.vector.tensor_scalar(sh[:], var4[:].bitcast(I32), 1, None,
                                op0=OP.logical_shift_right)
        sh2 = nstat.tile([128, 4], I32)
        nc.vector.tensor_scalar(sh2[:], sh[:], -1, 0x5F3759DF,
                                op0=OP.mult, op1=OP.add)
        y = nstat.tile([128, 4], FP)
        nc.vector.tensor_copy(y[:], sh2[:].bitcast(FP))
        t1 = nstat.tile([128, 4], FP)
        t2 = nstat.tile([128, 4], FP)
        for _ in range(3):
            nc.vector.tensor_mul(t1[:], y[:], y[:])
            nc.vector.tensor_mul(t2[:], t1[:], var4[:])
            nc.vector.tensor_scalar(t2[:], t2[:], -0.5, 1.5, op0=OP.mult, op1=OP.add)
            nc.vector.tensor_mul(y[:], y[:], t2[:])
        rstd4 = y

        # W2' = W2 * rstd (rows are h1 channels); z = max(h1, mu)
        for t in range(4):
            nc.gpsimd.tensor_scalar_mul(w2s_sb[:, t, :], w2_sb[:, t, :], rstd4[:, t:t + 1])
            nc.gpsimd.tensor_scalar_max(z_sb[:, t, :], h1_sb[:, t, :], mu4[:, t:t + 1])

        # bias correction: outb = b2 - W2'@mu
        mu4b = nstat.tile([128, 4], BF)
        nc.vector.tensor_copy(mu4b[:], mu4[:])
        bc_ps = w2p.tile([128, 2], FP, tag="bc")
        for oc in range(2):
            for kc in range(4):
                nc.tensor.matmul(
                    bc_ps[:, oc:oc + 1],
                    w2s_sb[:, kc, oc * 128:(oc + 1) * 128],
                    mu4b[:, kc:kc + 1],
                    start=(kc == 0), stop=(kc == 3),
                )
        outb = nstat.tile([128, 2], FP)
        nc.vector.tensor_tensor(out=outb[:], in0=b2_sb[:], in1=bc_ps[:], op=OP.subtract)

        # out = W2'@z + outb
        for oc in range(2):
            o_ps = w2p.tile([128, 2, 512], FP, tag="mm")
            for half in range(2):
                for kc in range(4):
                    nc.tensor.matmul(
                        o_ps[:, half, :],
                        w2s_sb[:, kc, oc * 128:(oc + 1) * 128],
                        z_sb[:, kc, half * 512:(half + 1) * 512],
                        start=(kc == 0), stop=(kc == 3),
                    )
            nc.vector.tensor_scalar_add(
                out_sb[:, oc, :].rearrange("p (a n) -> p a n", a=2),
                o_ps[:], outb[:, oc:oc + 1])

        nc.sync.dma_start(out=io["out"].rearrange("(c p) n -> p c n", p=128), in_=out_sb[:])


_BUILT = {}


def _build():
    if "nc" in _BUILT:
        return _BUILT["nc"]
    nc = bacc.Bacc("TRN2", target_bir_lowering=False, debug=False,
                   enable_asserts=True, num_devices=NCORES)
    io = {}
    io["xs"] = nc.dram_tensor("xs", [D, NS], BF, kind="ExternalInput").ap()
    io["src"] = nc.dram_tensor("src", [D, N], BF, kind="ExternalInput").ap()
    io["wqT"] = nc.dram_tensor("wqT", [D, D], BF, kind="ExternalInput").ap()
    io["wkT"] = nc.dram_tensor("wkT", [D, D], BF, kind="ExternalInput").ap()
    io["wvT"] = nc.dram_tensor("wvT", [D, D], BF, kind="ExternalInput").ap()
    io["w1xT"] = nc.dram_tensor("w1xT", [D, 2 * D], BF, kind="ExternalInput").ap()
    io["w1mT"] = nc.dram_tensor("w1mT", [D, 2 * D], BF, kind="ExternalInput").ap()
    io["w2T"] = nc.dram_tensor("w2T", [2 * D, D], BF, kind="ExternalInput").ap()
    io["bq"] = nc.dram_tensor("bq", [128, 2], FP, kind="ExternalInput").ap()
    io["b2"] = nc.dram_tensor("b2", [128, 2], FP, kind="ExternalInput").ap()
    io["out"] = nc.dram_tensor("out", [D, NS], FP, kind="ExternalOutput").ap()

    import contextlib
    with tile.TileContext(nc) as tc:
        with contextlib.ExitStack() as es:
            _emit(nc, tc, io, es)
    nc.compile()
    _BUILT["nc"] = nc
    return nc


def _prep_inputs(x, source, Wq, bq, Wk, bk, Wv, bv, Wm, bm, W1, b1, W2, b2):
    import ml_dtypes
    perm = np.array([4 * d + h for h in range(H) for d in range(DH)])
    bf = lambda a: np.ascontiguousarray(np.asarray(a, np.float32).astype(ml_dtypes.bfloat16))
    f32 = lambda a: np.ascontiguousarray(a, dtype=np.float32)

    W1x = W1[:, 0:D]
    W1m = np.asarray(W1[:, D:2 * D], np.float64) @ np.asarray(Wm, np.float64)
    W1mp = np.asarray(W1m, np.float32)[:, perm]

    shared = {
        "wqT": bf(Wq[perm, :].T),
        "wkT": bf(Wk[perm, :].T),
        "wvT": bf(Wv[perm, :].T),
        "w1xT": bf(W1x.T),
        "w1mT": bf(W1mp.T),
        "w2T": bf(W2.T),
        "bq": f32(bq[perm].reshape(2, 128).T),
        "b2": f32(b2.reshape(2, 128).T),
    }
    in_maps = []
    for core in range(NCORES):
        b, s = core // 4, core % 4
        m = dict(shared)
        m["xs"] = bf(x[b][:, s * NS:(s + 1) * NS])
        m["src"] = bf(source[b])
        in_maps.append(m)
    return in_maps


def run(inputs, **spmd_kwargs):
    """Build (cached), run on cores 0-7, return (full_output, BassKernelResults)."""
    nc = _build()
    in_maps = _prep_inputs(**inputs)
    res = bass_utils.run_bass_kernel_spmd(
        nc, in_maps, core_ids=list(range(NCORES)), **spmd_kwargs)
    full = np.empty((B, D, N), dtype=np.float32)
    for core in range(NCORES):
        b, s = core // 4, core % 4
        full[b][:, s * NS:(s + 1) * NS] = res.results[core]["out"]
    return full, res


def kernel(**inputs):
    full, _ = run(inputs)
    return full


# revision 23
# speedup vs baseline: 1.1483x; 1.1483x over previous
# Trainium2 Bass kernel for nn_AttentionalPropagation (B=2, D=256, N=M=4096, H=4).
#
# Sharding: 8 cores; each batch (B=2) owns 4 cores; each core computes a
# 1024-column sequence shard of the output end-to-end. k,v are computed
# redundantly per core from the full `source` of its batch. The only
# cross-core communication is an AllGather of the InstanceNorm partial
# (sum, sumsq) statistics within each 4-core batch group.
#
# Bias algebra (host-side folding):
#   bk drops: adds a per-n constant to scores -> softmax-invariant over m.
#   bv drops: adds bv to attn (sum prob = 1) -> folded via W1m@Wm@bv ... but
#             b1 itself cancels in InstanceNorm (mean-subtracted), so the
#             whole msg/cat bias chain (bm, bv, b1) vanishes.
#   Wm folds into W1: h1 = W1x@x + (W1m@Wm)@attn.
#   Only bq and b2 survive.
#
# Precision plan:
#   q,k fp8e4 + DoubleRow scores matmul ([32,2,*] layout, contraction 64).
#   exp: split ACT (native Exp -> fp8) / DVE (Schraudolph bit-trick:
#        uint8 = RNE(score*log2e + 56) == fp8e4 bits of exp(score/8)).
#   v fp8; attn matmul fp8 DoubleRow with 64 ones-columns appended so rows
#   64..127 of the accumulator hold Z (softmax denom) replicated -> parallel
#   reciprocal.
#   Everything else bf16 (h1, W2), fp32 accumulation.

import os
import numpy as np

import concourse.bass as bass  # noqa: F401
import concourse.tile as tile
import concourse.mybir as mybir
from concourse import bacc
from concourse import bass_utils

B, D, N = 2, 256, 4096
H, DH = 4, 64
NS = N // 4           # sequence shard per core
NCORES = 8
EPS = 1e-5
LOG2E = 1.4426950408889634
EXPC = 56.0           # fp8e4 exponent-bias shift: bits = RNE(s*log2e + 56)

FP = mybir.dt.float32
BF = mybir.dt.bfloat16
F8 = mybir.dt.float8e4
U8 = mybir.dt.uint8
I32 = mybir.dt.int32
U32 = mybir.dt.uint32
AX = mybir.AxisListType
OP = mybir.AluOpType
AF = mybir.ActivationFunctionType
DR = mybir.MatmulPerfMode.DoubleRow

_STAGE = os.environ.get("KSTAGE", "full")  # proj|attn|h1|nocc|full

# Per-(h,nc) exp group -> engine assignment: 16 groups of 2 m-chunks.
# ~62% ACT / 38% DVE, max run of 2 so the two engines ping-pong.
_EXP_ON_ACT = [True, False, True, True, False, True, True, False,
               True, True, False, True, False, True, True, False]
_ATT_LAG = 2  # attn-matmul emission lag (groups) behind scores+exp


def _emit(nc, tc, io, es):
    wpool = es.enter_context(tc.tile_pool(name="weights", bufs=1))
    apool = es.enter_context(tc.tile_pool(name="acts", bufs=1))

    # ---------- warm-up: ACT exp table load + PE warm stream ----------
    warm = apool.tile([128, 8], FP)
    nc.vector.memset(warm[:], 0.0)
    nc.scalar.activation(out=warm[:, 0:4], in_=warm[:, 4:8], func=AF.Exp, scale=1.0)

    # ---------- weight / bias / input loads ----------
    wq_sb = wpool.tile([128, 2, D], BF)
    nc.sync.dma_start(out=wq_sb[:], in_=io["wqT"].rearrange("(c p) o -> p c o", p=128))
    wk_sb = wpool.tile([128, 2, D], BF)
    nc.sync.dma_start(out=wk_sb[:], in_=io["wkT"].rearrange("(c p) o -> p c o", p=128))
    wv_sb = wpool.tile([128, 2, D], BF)
    nc.sync.dma_start(out=wv_sb[:], in_=io["wvT"].rearrange("(c p) o -> p c o", p=128))
    w1x_sb = wpool.tile([128, 2, 2 * D], BF)
    nc.gpsimd.dma_start(out=w1x_sb[:], in_=io["w1xT"].rearrange("(c p) o -> p c o", p=128))
    w1m_sb = wpool.tile([128, 2, 2 * D], BF)
    nc.gpsimd.dma_start(out=w1m_sb[:], in_=io["w1mT"].rearrange("(c p) o -> p c o", p=128))
    w2_sb = wpool.tile([128, 4, D], BF)
    nc.gpsimd.dma_start(out=w2_sb[:], in_=io["w2T"].rearrange("(c p) o -> p c o", p=128))

    bq_sb = wpool.tile([128, 2], FP)
    nc.sync.dma_start(out=bq_sb[:], in_=io["bq"][:])
    b2_sb = wpool.tile([128, 2], FP)
    nc.sync.dma_start(out=b2_sb[:], in_=io["b2"][:])

    xs_sb = apool.tile([128, 2, NS], BF)
    nc.sync.dma_start(out=xs_sb[:], in_=io["xs"].rearrange("(c p) n -> p c n", p=128))

    # ---------- persistent activation tiles ----------
    qf8 = apool.tile([128, 2, NS], U8)
    kf8 = apool.tile([128, 2, N], U8)
    q8 = apool.tile([32, H, 2, NS], U8)
    k8 = apool.tile([32, H, 2, N], U8)
    # v^T per head + 64 ones cols (Z replication), fp8: [m-part, h, p16, j, 128]
    vaT = apool.tile([128, H, 16, 2, 128], F8)
    exp8 = apool.tile([128, 32, 512], U8)
    attn_sb = apool.tile([128, 2, NS], BF)
    h1_sb = apool.tile([128, 4, NS], BF)
    h1n_sb = apool.tile([128, 4, NS], BF)
    out_sb = apool.tile([128, 2, NS], FP)
    stats_sb = apool.tile([128, 8], FP)

    # ones in cols 0..63 so Z lands at base partition 0 (custom-DVE recip
    # ignores input base_partition); v data in cols 64..127.
    for _h in range(H):
        for _j in range(2):
            nc.gpsimd.memset(vaT[:, _h, :, _j, 0:DH], 1.0)

    # ---------- phase 1: projections ----------
    with tc.tile_pool(name="srcp", bufs=1) as srcpool, \
         tc.tile_pool(name="pj", bufs=2, space="PSUM") as pj, \
         tc.tile_pool(name="vt", bufs=3, space="PSUM") as vtp:
        src_sb = srcpool.tile([128, 2, N], BF)
        nc.sync.dma_start(out=src_sb[:], in_=io["src"].rearrange("(c p) m -> p c m", p=128))

        # k = WkT.T @ src (no bias), per oc chunk; evac fp8, then per-head DMA
        # shuffle into the DoubleRow [32, 2, m] layout.
        for oc in range(2):
            for ns in range(N // 512 // 2):
                k_ps = pj.tile([128, 2, 512], FP, tag="pj")
                for half in range(2):
                    for ic in range(2):
                        nc.tensor.matmul(
                            k_ps[:, half, :],
                            wk_sb[:, ic, oc * 128:(oc + 1) * 128],
                            src_sb[:, ic, (ns * 2 + half) * 512:(ns * 2 + half + 1) * 512],
                            start=(ic == 0), stop=(ic == 1),
                        )
                nc.vector.tensor_copy(
                    kf8[:, oc, ns * 1024:(ns + 1) * 1024].bitcast(F8)
                    .rearrange("p (a n) -> p a n", a=2), k_ps[:])
            # shuffle: k8[p, h, j, m] = kf8[64*(h%2)+32*j+p, h//2, m]
            for m in range(2):
                h = 2 * oc + m
                for j in range(2):
                    nc.sync.dma_start(
                        out=k8[:, h, j, :],
                        in_=kf8[64 * m + 32 * j:64 * m + 32 * j + 32, oc, :])

        # q = WqT.T @ xs + bq; evac fp8 + shuffle
        for oc in range(2):
            q_ps = pj.tile([128, 2, 512], FP, tag="pj")
            for half in range(2):
                for ic in range(2):
                    nc.tensor.matmul(
                        q_ps[:, half, :],
                        wq_sb[:, ic, oc * 128:(oc + 1) * 128],
                        xs_sb[:, ic, half * 512:(half + 1) * 512],
                        start=(ic == 0), stop=(ic == 1),
                    )
            nc.vector.tensor_scalar_add(
                qf8[:, oc, :].bitcast(F8).rearrange("p (a n) -> p a n", a=2),
                q_ps[:], bq_sb[:, oc:oc + 1])
            for m in range(2):
                h = 2 * oc + m
                for j in range(2):
                    nc.sync.dma_start(
                        out=q8[:, h, j, :],
                        in_=qf8[64 * m + 32 * j:64 * m + 32 * j + 32, oc, :])

        # v^T directly transposed: out[m, c] = sum_i src[i, m] WvT[i, c]
        for mp in range(16):
            vt_ps = vtp.tile([128, 2, D], FP, tag="vt")
            for j in range(2):
                for ic in range(2):
                    nc.tensor.matmul(
                        vt_ps[:, j, :],
                        src_sb[:, ic, (2 * mp + j) * 128:(2 * mp + j + 1) * 128],
                        wv_sb[:, ic, :],
                        start=(ic == 0), stop=(ic == 1),
                    )
            # in order (j, h, d) -> out vaT[:, h, mp, j, 64:128] reordered (j, h, d)
            nc.vector.tensor_copy(
                vaT[:, :, mp, :, DH:128].rearrange("p h j d -> p j h d"),
                vt_ps[:].rearrange("p j (h d) -> p j h d", h=H))

    if _STAGE == "proj":
        nc.vector.tensor_copy(out_sb[:, 0, :], xs_sb[:, 0, :])
        nc.vector.tensor_copy(out_sb[:, 1, :], xs_sb[:, 1, :])
        nc.sync.dma_start(out=io["out"].rearrange("(c p) n -> p c n", p=128), in_=out_sb[:])
        return

    # ---------- phase 2+3 interleaved: attention then h1 per nc chunk ----------
    # Emission is software-pipelined: the attn DR matmul for group g is
    # emitted _ATT_LAG groups late so the PE's in-order queue never stalls
    # waiting on exp(g) -- the next groups' scores matmuls run first and the
    # two exp engines (ACT/DVE) stay saturated.
    from collections import deque
    with tc.tile_pool(name="sc", bufs=1, space="PSUM") as scp, \
         tc.tile_pool(name="at", bufs=2, space="PSUM") as atp, \
         tc.tile_pool(name="mm", bufs=1, space="PSUM") as mmp, \
         tc.tile_pool(name="nrm", bufs=2) as nrm:
        sc = scp.tile([128, 4, 512], FP)  # quartered: 2-mc groups double-buffered
        pend = deque()

        def emit_at(e):
            h, g, n0, at_ps = e
            nc.tensor.matmul(
                at_ps[:],
                vaT[:, h, g, :, :],
                exp8[:, 2 * g:2 * g + 2, :].bitcast(F8),
                start=(g == 0), stop=(g == 15),
                perf_mode=DR,
            )
            if g == 15:
                rzb = nrm.tile([DH, 512], FP, tag="rzb")
                nc.vector.reciprocal_approx_fast(rzb[:], at_ps[0:DH, :])
                bp = 64 * (h % 2)
                nc.vector.tensor_tensor(
                    out=attn_sb[bp:bp + DH, h // 2, n0:n0 + 512],
                    in0=at_ps[DH:128, :], in1=rzb[:], op=OP.mult)

        for nch in range(2):
            n0 = nch * 512
            for h in range(4):
                at_ps = atp.tile([128, 512], FP, tag="at")
                for g in range(16):      # groups of 2 m-chunks
                    q2 = 2 * (g % 2)
                    for j2 in range(2):  # m-chunk within group
                        mc = 2 * g + j2
                        nc.tensor.matmul(
                            sc[:, q2 + j2, :],
                            k8[:, h, :, mc * 128:(mc + 1) * 128].bitcast(F8),
                            q8[:, h, :, n0:n0 + 512].bitcast(F8),
                            start=True, stop=True,
                            perf_mode=DR,
                        )
                    if _EXP_ON_ACT[g]:
                        nc.scalar.activation(
                            out=exp8[:, 2 * g:2 * g + 2, :].bitcast(F8),
                            in_=sc[:, q2:q2 + 2, :], func=AF.Exp, scale=0.125)
                    else:
                        nc.vector.tensor_scalar(
                            exp8[:, 2 * g:2 * g + 2, :],
                            sc[:, q2:q2 + 2, :], LOG2E, EXPC,
                            op0=OP.mult, op1=OP.add)
                    pend.append((h, g, n0, at_ps))
                    while len(pend) > _ATT_LAG:
                        emit_at(pend.popleft())
            while pend:
                emit_at(pend.popleft())

            if _STAGE == "attn" and nch == 1:
                nc.vector.tensor_copy(out_sb[:, 0, :], attn_sb[:, 0, :])
                nc.vector.tensor_copy(out_sb[:, 1, :], attn_sb[:, 1, :])
                nc.sync.dma_start(out=io["out"].rearrange("(c p) n -> p c n", p=128), in_=out_sb[:])
                return

            # h1 (raw, biasless) = W1x@xs + (W1m@Wm)@attn for this nc chunk
            for op2 in range(2):  # oc pairs
                h_ps = mmp.tile([128, 2, 512], FP, tag="mm")
                for oc2 in range(2):
                    oc = 2 * op2 + oc2
                    for ic in range(2):
                        nc.tensor.matmul(
                            h_ps[:, oc2, :],
                            w1x_sb[:, ic, oc * 128:(oc + 1) * 128],
                            xs_sb[:, ic, n0:n0 + 512],
                            start=(ic == 0), stop=False,
                        )
                    for ic in range(2):
                        nc.tensor.matmul(
                            h_ps[:, oc2, :],
                            w1m_sb[:, ic, oc * 128:(oc + 1) * 128],
                            attn_sb[:, ic, n0:n0 + 512],
                            start=False, stop=(ic == 1),
                        )
                nc.vector.tensor_copy(
                    h1_sb[:, 2 * op2:2 * op2 + 2, n0:n0 + 512], h_ps[:])

    if _STAGE == "h1":
        nc.vector.tensor_copy(out_sb[:, 0, :], h1_sb[:, 0, :])
        nc.vector.tensor_copy(out_sb[:, 1, :], h1_sb[:, 1, :])
        nc.sync.dma_start(out=io["out"].rearrange("(c p) n -> p c n", p=128), in_=out_sb[:])
        return

    # ---------- phase 4: stats, gather, norm, W2 ----------
    with tc.tile_pool(name="dram", bufs=1, space="DRAM") as dram, \
         tc.tile_pool(name="nstat", bufs=1) as nstat, \
         tc.tile_pool(name="w2p", bufs=2, space="PSUM") as w2p:
        # per-core partial (sum, sumsq) via bn_stats/bn_aggr (mean, biased var)
        for t in range(4):
            bst = nstat.tile([128, 2, 6], FP, tag="bst")
            for g in range(2):
                nc.vector.bn_stats(out=bst[:, g, :], in_=h1_sb[:, t, g * 512:(g + 1) * 512])
            mv = nstat.tile([128, 2], FP, tag="mv")
            nc.vector.bn_aggr(out=mv[:], in_=bst[:])
            nc.vector.tensor_scalar_mul(stats_sb[:, t:t + 1], mv[:, 0:1], float(NS))
            msq = nstat.tile([128, 1], FP, tag="msq")
            nc.vector.tensor_mul(msq[:], mv[:, 0:1], mv[:, 0:1])
            msq2 = nstat.tile([128, 1], FP, tag="msq2")
            nc.vector.tensor_add(msq2[:], mv[:, 1:2], msq[:])
            nc.vector.tensor_scalar_mul(stats_sb[:, 4 + t:5 + t], msq2[:], float(NS))

        sred = nstat.tile([128, 8], FP)
        if _STAGE == "nocc":
            nc.vector.tensor_scalar_mul(sred[:], stats_sb[:], 4.0)
        else:
            cc_in = dram.tile([128, 8], FP)
            cc_out = dram.tile([4, 128, 8], FP)
            nc.sync.dma_start(out=cc_in[:], in_=stats_sb[:])
            nc.gpsimd.collective_compute(
                "AllGather", OP.bypass,
                replica_groups=[[0, 1, 2, 3], [4, 5, 6, 7]],
                ins=[cc_in[:].opt()], outs=[cc_out[:].opt()],
            )
            gat = nstat.tile([128, 4, 8], FP)
            nc.sync.dma_start(out=gat[:], in_=cc_out[:].rearrange("s p f -> p s f"))
            s01 = nstat.tile([128, 2, 8], FP)
            nc.vector.tensor_add(s01[:], gat[:, 0:2, :], gat[:, 2:4, :])
            nc.vector.tensor_add(sred[:], s01[:, 0, :], s01[:, 1, :])

        # mu = sum/N ; var = sumsq/N - mu^2 ; rstd = rsqrt(var+eps) (Newton)
        mu4 = nstat.tile([128, 4], FP)
        nc.vector.tensor_scalar_mul(mu4[:], sred[:, 0:4], 1.0 / N)
        var4 = nstat.tile([128, 4], FP)
        nc.vector.tensor_scalar_mul(var4[:], sred[:, 4:8], 1.0 / N)
        musq = nstat.tile([128, 4], FP)
        nc.vector.tensor_mul(musq[:], mu4[:], mu4[:])
        nc.vector.tensor_tensor(out=var4[:], in0=var4[:], in1=musq[:], op=OP.subtract)
        nc.vector.tensor_scalar_add(var4[:], var4[:], EPS)
        # Newton rsqrt: y0 = bits(0x5F3759DF - (u>>1)); y_{k+1} = y(1.5 - 0.5 x y^2)
        sh = nstat.tile([128, 4], I32)
        nc.vector.tensor_scalar(sh[:], var4[:].bitcast(I32), 1, None,
                                op0=OP.logical_shift_right)
        sh2 = nstat.tile([128, 4], I32)
        nc.vector.tensor_scalar(sh2[:], sh[:], -1, 0x5F3759DF,
                                op0=OP.mult, op1=OP.add)
        y = nstat.tile([128, 4], FP)
        nc.vector.tensor_copy(y[:], sh2[:].bitcast(FP))
        t1 = nstat.tile([128, 4], FP)
        t2 = nstat.tile([128, 4], FP)
        for _ in range(3):
            nc.vector.tensor_mul(t1[:], y[:], y[:])
            nc.vector.tensor_mul(t2[:], t1[:], var4[:])
            nc.vector.tensor_scalar(t2[:], t2[:], -0.5, 1.5, op0=OP.mult, op1=OP.add)
            nc.vector.tensor_mul(y[:], y[:], t2[:])
        rstd4 = y

        nb4 = nstat.tile([128, 4], FP)
        nc.vector.tensor_mul(nb4[:], mu4[:], rstd4[:])
        nc.vector.tensor_scalar_mul(nb4[:], nb4[:], -1.0)

        # h = relu(h1 * rstd - mu * rstd) on ACT (relu is in every table set)
        for t in range(4):
            nc.scalar.activation(
                out=h1n_sb[:, t, :], in_=h1_sb[:, t, :], func=AF.Relu,
                bias=nb4[:, t:t + 1], scale=rstd4[:, t:t + 1])

        # out = W2T.T @ h + b2
        for oc in range(2):
            o_ps = w2p.tile([128, 2, 512], FP, tag="mm")
            for half in range(2):
                for kc in range(4):
                    nc.tensor.matmul(
                        o_ps[:, half, :],
                        w2_sb[:, kc, oc * 128:(oc + 1) * 128],
                        h1n_sb[:, kc, half * 512:(half + 1) * 512],
                        start=(kc == 0), stop=(kc == 3),
                    )
            nc.vector.tensor_scalar_add(
                out_sb[:, oc, :].rearrange("p (a n) -> p a n", a=2),
                o_ps[:], b2_sb[:, oc:oc + 1])

        nc.sync.dma_start(out=io["out"].rearrange("(c p) n -> p c n", p=128), in_=out_sb[:])


_BUILT = {}


def _build():
    if "nc" in _BUILT:
        return _BUILT["nc"]
    nc = bacc.Bacc("TRN2", target_bir_lowering=False, debug=False,
                   enable_asserts=True, num_devices=NCORES)
    io = {}
    io["xs"] = nc.dram_tensor("xs", [D, NS], BF, kind="ExternalInput").ap()
    io["src"] = nc.dram_tensor("src", [D, N], BF, kind="ExternalInput").ap()
    io["wqT"] = nc.dram_tensor("wqT", [D, D], BF, kind="ExternalInput").ap()
    io["wkT"] = nc.dram_tensor("wkT", [D, D], BF, kind="ExternalInput").ap()
    io["wvT"] = nc.dram_tensor("wvT", [D, D], BF, kind="ExternalInput").ap()
    io["w1xT"] = nc.dram_tensor("w1xT", [D, 2 * D], BF, kind="ExternalInput").ap()
    io["w1mT"] = nc.dram_tensor("w1mT", [D, 2 * D], BF, kind="ExternalInput").ap()
    io["w2T"] = nc.dram_tensor("w2T", [2 * D, D], BF, kind="ExternalInput").ap()
    io["bq"] = nc.dram_tensor("bq", [128, 2], FP, kind="ExternalInput").ap()
    io["b2"] = nc.dram_tensor("b2", [128, 2], FP, kind="ExternalInput").ap()
    io["out"] = nc.dram_tensor("out", [D, NS], FP, kind="ExternalOutput").ap()

    import contextlib
    with tile.TileContext(nc) as tc:
        with contextlib.ExitStack() as es:
            _emit(nc, tc, io, es)
    nc.compile()
    _BUILT["nc"] = nc
    return nc


def _prep_inputs(x, source, Wq, bq, Wk, bk, Wv, bv, Wm, bm, W1, b1, W2, b2):
    import ml_dtypes
    perm = np.array([4 * d + h for h in range(H) for d in range(DH)])
    bf = lambda a: np.ascontiguousarray(np.asarray(a, np.float32).astype(ml_dtypes.bfloat16))
    f32 = lambda a: np.ascontiguousarray(a, dtype=np.float32)

    W1x = W1[:, 0:D]
    W1m = np.asarray(W1[:, D:2 * D], np.float64) @ np.asarray(Wm, np.float64)
    W1mp = np.asarray(W1m, np.float32)[:, perm]

    shared = {
        "wqT": bf(Wq[perm, :].T),
        "wkT": bf(Wk[perm, :].T),
        "wvT": bf(Wv[perm, :].T),
        "w1xT": bf(W1x.T),
        "w1mT": bf(W1mp.T),
        "w2T": bf(W2.T),
        "bq": f32(bq[perm].reshape(2, 128).T),
        "b2": f32(b2.reshape(2, 128).T),
    }
    in_maps = []
    for core in range(NCORES):
        b, s = core // 4, core % 4
        m = dict(shared)
        m["xs"] = bf(x[b][:, s * NS:(s + 1) * NS])
        m["src"] = bf(source[b])
        in_maps.append(m)
    return in_maps


def run(inputs, **spmd_kwargs):
    """Build (cached), run on cores 0-7, return (full_output, BassKernelResults)."""
    nc = _build()
    in_maps = _prep_inputs(**inputs)
    res = bass_utils.run_bass_kernel_spmd(
        nc, in_maps, core_ids=list(range(NCORES)), **spmd_kwargs)
    full = np.empty((B, D, N), dtype=np.float32)
    for core in range(NCORES):
        b, s = core // 4, core % 4
        full[b][:, s * NS:(s + 1) * NS] = res.results[core]["out"]
    return full, res


def kernel(**inputs):
    full, _ = run(inputs)
    return full
